# revision 7
# baseline (speedup 1.0000x reference)
"""Multi-head causal attention (B=4, L=2048, D=1024, H=16) on 8 TRN2 NeuronCores.

Sharding: core c handles batch b = c//2 and head-group hg = c%2 (8 heads, 512
dims). Each core computes Q/K/V projections for its heads, causal attention,
and a partial output projection (its 512 input dims of Wo). Host sums the two
partials per batch.

v3: inputs arrive pre-transposed and pre-cast to fp16 from the host (x^T,
Wq^T, Wk^T/8, Wv^T, Wo^T) — no on-device input transposes. Scores near the
causal diagonal are trimmed to 128-column granularity, masking is one
triangular [128,128] multiply per diagonal block, and the softmax epilogue is
one reciprocal + one broadcast multiply per (head, chunk). The attention
stream is software-pipelined one head deep: head h's AV/epilogue work is
interleaved between head h+1's score matmuls so the scalar engine (exp) never
starves — the kernel's back half is exp-throughput-bound. Projection work for
the next chunk and the output projection are smeared into remaining PE slack.
"""
import sys

sys.path.insert(0, "/opt/trn_rl_repo")

import numpy as np

import concourse.bass as bass
import concourse.mybir as mybir
import concourse.tile as tile
from concourse import bacc
from concourse.masks import make_identity

F32 = mybir.dt.float32
F16 = mybir.dt.float16
MM = F16
AF = mybir.ActivationFunctionType

B, L, D, H = 4, 2048, 1024, 16
DK = 64
E = 512
NL = L // 128
ND = D // 128
NE = E // 128
NJ = L // 512
NK = L // 128
NDO = E // 128

_CACHE = {}


def build_program():
    nc = bacc.Bacc("TRN2", target_bir_lowering=False, debug=False, num_devices=8)

    xTd = nc.dram_tensor("xT", [D, L], F16, kind="ExternalInput")
    wqT = nc.dram_tensor("wqT", [D, E], F16, kind="ExternalInput")
    wkT = nc.dram_tensor("wkT", [D, E], F16, kind="ExternalInput")
    wvT = nc.dram_tensor("wvT", [D, E], F16, kind="ExternalInput")
    woT = nc.dram_tensor("woT", [E, D], F16, kind="ExternalInput")
    trid = nc.dram_tensor("tri", [128, 128], F16, kind="ExternalInput")
    out = nc.dram_tensor("out", [L, D], F32, kind="ExternalOutput")

    with tile.TileContext(nc) as tc:
        with (
            tc.tile_pool(name="const", bufs=1) as constp,
            tc.tile_pool(name="big", bufs=1) as bigp,
            tc.tile_pool(name="qtc", bufs=2) as qtcp,
            tc.tile_pool(name="ptp", bufs=3) as ptp,
            tc.tile_pool(name="smallp", bufs=2) as smallp,
            tc.tile_pool(name="attsbp", bufs=2) as attsbp,
            tc.tile_pool(name="psM", bufs=2, space="PSUM") as psM,
            tc.tile_pool(name="psP", bufs=2, space="PSUM") as psP,
            tc.tile_pool(name="psS", bufs=2, space="PSUM") as psS,
        ):
            ident_h = constp.tile([128, 128], F16)
            make_identity(nc, ident_h[:])
            tri = constp.tile([128, 128], F16)
            nc.sync.dma_start(tri[:], trid[:])

            xT = bigp.tile([128, ND, L], MM)       # x^T  [d-in-tile, d-tile, l]
            WTq = bigp.tile([128, ND, E], MM)      # Wq^T [d-in-tile, d-tile, e]
            WTk = bigp.tile([128, ND, E], MM)
            WTv = bigp.tile([128, ND, E], MM)
            KT = bigp.tile([128, NE, L], MM)       # K^T  [dk (2 heads), e-tile, k]
            attT = bigp.tile([128, NDO, L], MM)
            WoT = bigp.tile([128, NDO, D], MM)
            Vaug = bigp.tile([128, NK, 8, 65], MM)  # V natural per (k-tile, head) + ones

            nc.vector.memset(Vaug[:, :, :, 64:65], 1.0)

            # ---------- input DMAs, dependency order ----------
            def dma_w(dst, src):
                for dt in range(ND):
                    nc.sync.dma_start(
                        dst[:, dt, :], src[dt * 128:(dt + 1) * 128, :]
                    )

            def dma_x(jc):
                for dt in range(ND):
                    nc.sync.dma_start(
                        xT[:, dt, jc * 512:(jc + 1) * 512],
                        xTd[dt * 128:(dt + 1) * 128, jc * 512:(jc + 1) * 512],
                    )

            dma_w(WTk, wkT)
            dma_x(0)
            dma_w(WTq, wqT)
            dma_w(WTv, wvT)
            for jc in range(1, NJ):
                dma_x(jc)
            for et8 in range(NDO):
                nc.sync.dma_start(
                    WoT[:, et8, :], woT[et8 * 128:(et8 + 1) * 128, :]
                )

            # ---------- emission helpers ----------
            def k_proj_group(et, jc):
                def gen():
                    pp = psP.tile([128, 512], F32, tag="pp", name="pp")
                    for dt in range(ND):
                        yield lambda dt=dt, pp=pp: nc.tensor.matmul(
                            pp[:],
                            WTk[:, dt, et * 128:(et + 1) * 128],
                            xT[:, dt, jc * 512:(jc + 1) * 512],
                            start=(dt == 0),
                            stop=(dt == ND - 1),
                        )
                    yield lambda pp=pp: nc.vector.tensor_copy(
                        KT[:, et, jc * 512:(jc + 1) * 512], pp[:]
                    )
                return gen()

            def v_proj_group(lt):
                def gen():
                    pp = psP.tile([128, 512], F32, tag="pp", name="pp")
                    for dt in range(ND):
                        yield lambda dt=dt, pp=pp: nc.tensor.matmul(
                            pp[:],
                            xT[:, dt, lt * 128:(lt + 1) * 128],
                            WTv[:, dt, :],
                            start=(dt == 0),
                            stop=(dt == ND - 1),
                        )
                    yield lambda pp=pp: nc.vector.tensor_copy(
                        Vaug[:, lt, :, 0:64], pp[:]
                    )
                return gen()

            def q_proj_group(j, qtile, et):
                def gen():
                    pp = psP.tile([128, 512], F32, tag="pp", name="pp")
                    for dt in range(ND):
                        yield lambda dt=dt, pp=pp: nc.tensor.matmul(
                            pp[:],
                            WTq[:, dt, et * 128:(et + 1) * 128],
                            xT[:, dt, j * 512:(j + 1) * 512],
                            start=(dt == 0),
                            stop=(dt == ND - 1),
                        )
                    yield lambda pp=pp: nc.vector.tensor_copy(qtile[:, et, :], pp[:])
                return gen()

            def out_proj_group(lt, ec):
                def gen():
                    op = psP.tile([128, 512], F32, tag="pp", name="op")
                    for dt in range(NDO):
                        yield lambda dt=dt, op=op: nc.tensor.matmul(
                            op[:],
                            attT[:, dt, lt * 128:(lt + 1) * 128],
                            WoT[:, dt, ec * 512:(ec + 1) * 512],
                            start=(dt == 0),
                            stop=(dt == NDO - 1),
                        )
                    def tail(op=op):
                        ot = smallp.tile([128, 512], F32, tag="ot", name="ot")
                        nc.vector.tensor_copy(ot[:], op[:])
                        nc.sync.dma_start(
                            out[lt * 128:(lt + 1) * 128, ec * 512:(ec + 1) * 512],
                            ot[:],
                        )
                    yield tail
                return gen()

            def chain(gens):
                for g in gens:
                    yield from g

            def drain(it, n):
                k = 0
                for f in it:
                    f()
                    k += 1
                    if k >= n:
                        return

            # ---------- one-head-deep AV/epilogue pipeline state ----------
            # pend = (j, h, PT, QTc) whose AV has not run yet
            state = {"pend": None, "att_pair": None}

            def av_steps(jh):
                """Yield closures: 4 AV accumulation groups, then the
                epilogue (reciprocal + normalize + pair transposes)."""
                j, h, PT = jh
                hp = (h % 2) * 64
                hb = h // 2
                if h % 2 == 0:
                    state["att_pair"] = attsbp.tile(
                        [128, 4, 128], MM, tag="apair", name="apair"
                    )
                att_pair = state["att_pair"]
                att_ps = psM.tile([128, 4, 65], F32, tag="m", name="att_ps")

                def group(qt):
                    nq = 4 * j + qt + 1
                    for kt in range(nq):
                        nc.tensor.matmul(
                            att_ps[:, qt, :],
                            PT[:, kt, qt * 128:(qt + 1) * 128],
                            Vaug[:, kt, h, 0:65],
                            start=(kt == 0),
                            stop=(kt == nq - 1),
                        )

                for qt in range(4):
                    yield lambda qt=qt: group(qt)

                def epilogue():
                    rc = smallp.tile([128, 4, 1], F32, tag="rc", name="rc")
                    nc.vector.reciprocal(rc[:], att_ps[:, :, 64:65])
                    nc.vector.tensor_mul(
                        att_pair[:, :, hp:hp + 64],
                        att_ps[:, :, 0:64],
                        rc[:, :, 0:1].to_broadcast((128, 4, 64)),
                    )
                    if h % 2 == 1:
                        for qt in range(4):
                            tpa = psM.tile([128, 128], F16, tag="m", name="tpa")
                            nc.tensor.transpose(
                                tpa[:], att_pair[:, qt, :], ident_h[:]
                            )
                            nc.vector.tensor_copy(
                                attT[:, hb, j * 512 + qt * 128:j * 512 + (qt + 1) * 128],
                                tpa[:],
                            )
                yield epilogue

            # ---------- prologue: K, Q and V of chunk 0 ----------
            for et in range(NE):
                drain(k_proj_group(et, 0), 99)
            qtiles = {0: qtcp.tile([128, NE, 512], MM, tag="qt", name="qt0")}
            for et in range(NE):
                drain(q_proj_group(0, qtiles[0], et), 99)
            for lt in range(4):
                drain(v_proj_group(lt), 99)

            # ---------- chunk loop ----------
            for j in range(NJ):
                gens = []
                if j + 1 < NJ:
                    qtiles[j + 1] = qtcp.tile(
                        [128, NE, 512], MM, tag="qt", name=f"qt{j+1}"
                    )
                    for et in range(NE):
                        gens.append(k_proj_group(et, j + 1))
                    for et in range(NE):
                        gens.append(q_proj_group(j + 1, qtiles[j + 1], et))
                    for lt in range(4 * (j + 1), 4 * (j + 1) + 4):
                        gens.append(v_proj_group(lt))
                n_fill_items = 9 * 12 if j + 1 < NJ else 0
                op_gens = []
                if j >= 1:
                    # output projection for chunk j-1's rows (gated on the
                    # carried-over pending head finishing its attT writes)
                    for lt in range(4 * (j - 1), 4 * (j - 1) + 4):
                        for ec in range(2):
                            op_gens.append(out_proj_group(lt, ec))
                n_op_items = 5 * len(op_gens)
                filler = chain(gens)
                op_filler = chain(op_gens)
                nkt = 4 * (j + 1)
                QTc = qtiles[j]
                n_slots = 8 * (nkt // 2)
                n_op_slots = 7 * (nkt // 2)
                fill_acc = 0.0
                fill_done = 0
                op_acc = 0.0
                op_done = 0
                for h in range(8):
                    hp = (h % 2) * 64
                    hb = h // 2
                    PT = ptp.tile([128, NK, 512], MM, tag="pt", name="pt")
                    pend_steps = (
                        av_steps(state["pend"]) if state["pend"] is not None else None
                    )
                    n_pend = 5 if pend_steps is not None else 0
                    pend_acc = 0.0
                    pend_done = 0
                    slots = nkt // 2
                    for sl in range(slots):
                        s_ps = psS.tile([128, 2, 512], F32, tag="s", name="s_ps")
                        if sl < slots - 2:
                            # full-width k-tile pair
                            for half in range(2):
                                kt = 2 * sl + half
                                nc.tensor.matmul(
                                    s_ps[:, half, :],
                                    KT[hp:hp + 64, hb, kt * 128:(kt + 1) * 128],
                                    QTc[hp:hp + 64, hb, :],
                                )
                            nc.scalar.activation(
                                PT[:, 2 * sl:2 * sl + 2, :], s_ps[:], AF.Exp
                            )
                        else:
                            # diagonal pair at 128-col causal granularity
                            mp = sl - (slots - 2)
                            for half in range(2):
                                m = 2 * mp + half
                                kt = nkt - 4 + m
                                nc.tensor.matmul(
                                    s_ps[:, half, 128 * m:512],
                                    KT[hp:hp + 64, hb, kt * 128:(kt + 1) * 128],
                                    QTc[hp:hp + 64, hb, 128 * m:512],
                                )
                            for half in range(2):
                                m = 2 * mp + half
                                kt = nkt - 4 + m
                                nc.scalar.activation(
                                    PT[:, kt, 128 * m:512],
                                    s_ps[:, half, 128 * m:512],
                                    AF.Exp,
                                )
                                nc.vector.tensor_mul(
                                    PT[:, kt, 128 * m:128 * (m + 1)],
                                    PT[:, kt, 128 * m:128 * (m + 1)],
                                    tri[:],
                                )
                        # interleave pending head's AV/epilogue
                        pend_acc += n_pend / slots
                        take = int(pend_acc) - pend_done
                        if take > 0 and pend_steps is not None:
                            drain(pend_steps, take)
                            pend_done += take
                        # smear next-chunk projection filler
                        fill_acc += n_fill_items / n_slots
                        take = int(fill_acc) - fill_done
                        if take > 0:
                            drain(filler, take)
                            fill_done += take
                        # output projection only once the cross-chunk pending
                        # head (which writes the last attT stripe) is done
                        if h >= 1:
                            op_acc += n_op_items / n_op_slots
                            take = int(op_acc) - op_done
                            if take > 0:
                                drain(op_filler, take)
                                op_done += take
                    if pend_steps is not None:
                        drain(pend_steps, 99)
                    state["pend"] = (j, h, PT)
                drain(filler, 10 ** 9)
                drain(op_filler, 10 ** 9)

            # final pending head + last chunk's output projection
            drain(av_steps(state["pend"]), 99)
            for lt in range(4 * (NJ - 1), 4 * (NJ - 1) + 4):
                for ec in range(2):
                    drain(out_proj_group(lt, ec), 99)

    nc.compile()
    return nc


def _get_program():
    if "nc" not in _CACHE:
        _CACHE["nc"] = build_program()
    return _CACHE["nc"]


def make_in_maps(x, Wq, Wk, Wv, Wo):
    x = np.asarray(x, dtype=np.float32)
    Wq = np.asarray(Wq, dtype=np.float32)
    Wk = np.asarray(Wk, dtype=np.float32)
    Wv = np.asarray(Wv, dtype=np.float32)
    Wo = np.asarray(Wo, dtype=np.float32)
    tri = (np.arange(128)[None, :] >= np.arange(128)[:, None]).astype(np.float16)
    in_maps = []
    for c in range(8):
        b, hg = c // 2, c % 2
        sl = slice(hg * E, (hg + 1) * E)
        in_maps.append(
            {
                "xT": x[b].T.astype(np.float16),
                "wqT": Wq[sl].T.astype(np.float16),
                "wkT": (Wk[sl] * 0.125).T.astype(np.float16),
                "wvT": Wv[sl].T.astype(np.float16),
                "woT": Wo[:, sl].T.astype(np.float16),
                "tri": tri,
            }
        )
    return in_maps


def kernel(x, Wq, Wk, Wv, Wo, **run_kwargs):
    from concourse import bass_utils

    nc = _get_program()
    in_maps = make_in_maps(x, Wq, Wk, Wv, Wo)
    res = bass_utils.run_bass_kernel_spmd(
        nc, in_maps, core_ids=list(range(8)), **run_kwargs
    )
    o = np.empty((B, L, D), np.float32)
    for b in range(B):
        o[b] = res.results[2 * b]["out"] + res.results[2 * b + 1]["out"]
    _CACHE["last_result"] = res
    return o


# revision 12
# speedup vs baseline: 1.0285x; 1.0285x over previous
"""Multi-head causal attention (B=4, L=2048, D=1024, H=16) on 8 TRN2 NeuronCores.

Sharding: core c handles batch b = c//2 and head-group hg = c%2 (8 heads, 512
dims). Each core computes Q/K/V projections for its heads, causal attention,
and a partial output projection (its 512 input dims of Wo). Host sums the two
partials per batch.

v3: inputs arrive pre-transposed and pre-cast to fp16 from the host (x^T,
Wq^T, Wk^T/8, Wv^T, Wo^T) — no on-device input transposes. Scores near the
causal diagonal are trimmed to 128-column granularity, masking is one
triangular [128,128] multiply per diagonal block, and the softmax epilogue is
one reciprocal + one broadcast multiply per (head, chunk). The attention
stream is software-pipelined one head deep: head h's AV/epilogue work is
interleaved between head h+1's score matmuls so the scalar engine (exp) never
starves — the kernel's back half is exp-throughput-bound. Projection work for
the next chunk and the output projection are smeared into remaining PE slack.
"""
import sys

sys.path.insert(0, "/opt/trn_rl_repo")

import numpy as np

import concourse.bass as bass
import concourse.mybir as mybir
import concourse.tile as tile
from concourse import bacc
from concourse.masks import make_identity

F32 = mybir.dt.float32
F16 = mybir.dt.float16
MM = F16
AF = mybir.ActivationFunctionType

B, L, D, H = 4, 2048, 1024, 16
DK = 64
E = 512
NL = L // 128
ND = D // 128
NE = E // 128
NJ = L // 512
NK = L // 128
NDO = E // 128

_CACHE = {}


def build_program():
    nc = bacc.Bacc("TRN2", target_bir_lowering=False, debug=False, num_devices=8)

    xTd = nc.dram_tensor("xT", [D, L], F16, kind="ExternalInput")
    wqT = nc.dram_tensor("wqT", [D, E], F16, kind="ExternalInput")
    wkT = nc.dram_tensor("wkT", [D, E], F16, kind="ExternalInput")
    wvT = nc.dram_tensor("wvT", [D, E], F16, kind="ExternalInput")
    woT = nc.dram_tensor("woT", [E, D], F16, kind="ExternalInput")
    trid = nc.dram_tensor("tri", [128, 128], F16, kind="ExternalInput")
    out = nc.dram_tensor("out", [L, D], F16, kind="ExternalOutput")

    with tile.TileContext(nc) as tc:
        with (
            tc.tile_pool(name="const", bufs=1) as constp,
            tc.tile_pool(name="big", bufs=1) as bigp,
            tc.tile_pool(name="qtc", bufs=2) as qtcp,
            tc.tile_pool(name="ptp", bufs=3) as ptp,
            tc.tile_pool(name="smallp", bufs=2) as smallp,
            tc.tile_pool(name="attsbp", bufs=2) as attsbp,
            tc.tile_pool(name="psM", bufs=2, space="PSUM") as psM,
            tc.tile_pool(name="psP", bufs=2, space="PSUM") as psP,
            tc.tile_pool(name="psS", bufs=2, space="PSUM") as psS,
        ):
            ident_h = constp.tile([128, 128], F16)
            make_identity(nc, ident_h[:])
            tri = constp.tile([128, 128], F16)
            nc.sync.dma_start(tri[:], trid[:])

            xT = bigp.tile([128, ND, L], MM)       # x^T  [d-in-tile, d-tile, l]
            WTq = bigp.tile([128, ND, E], MM)      # Wq^T [d-in-tile, d-tile, e]
            WTk = bigp.tile([128, ND, E], MM)
            WTv = bigp.tile([128, ND, E], MM)
            KT = bigp.tile([128, NE, L], MM)       # K^T  [dk (2 heads), e-tile, k]
            attT = bigp.tile([128, NDO, L], MM)
            WoT = bigp.tile([128, NDO, D], MM)
            Vaug = bigp.tile([128, NK, 8, 65], MM)  # V natural per (k-tile, head) + ones

            nc.vector.memset(Vaug[:, :, :, 64:65], 1.0)

            # ---------- input DMAs, dependency order ----------
            def dma_w(dst, src):
                for dt in range(ND):
                    nc.sync.dma_start(
                        dst[:, dt, :], src[dt * 128:(dt + 1) * 128, :]
                    )

            def dma_x(jc):
                for dt in range(ND):
                    nc.sync.dma_start(
                        xT[:, dt, jc * 512:(jc + 1) * 512],
                        xTd[dt * 128:(dt + 1) * 128, jc * 512:(jc + 1) * 512],
                    )

            dma_w(WTk, wkT)
            dma_x(0)
            dma_w(WTq, wqT)
            dma_w(WTv, wvT)
            for jc in range(1, NJ):
                dma_x(jc)
            for et8 in range(NDO):
                nc.sync.dma_start(
                    WoT[:, et8, :], woT[et8 * 128:(et8 + 1) * 128, :]
                )

            # ---------- emission helpers ----------
            def k_proj_group(et, jc):
                def gen():
                    pp = psP.tile([128, 512], F32, tag="pp", name="pp")
                    for dt in range(ND):
                        yield lambda dt=dt, pp=pp: nc.tensor.matmul(
                            pp[:],
                            WTk[:, dt, et * 128:(et + 1) * 128],
                            xT[:, dt, jc * 512:(jc + 1) * 512],
                            start=(dt == 0),
                            stop=(dt == ND - 1),
                        )
                    yield lambda pp=pp: nc.vector.tensor_copy(
                        KT[:, et, jc * 512:(jc + 1) * 512], pp[:]
                    )
                return gen()

            def v_proj_group(lt):
                def gen():
                    pp = psP.tile([128, 512], F32, tag="pp", name="pp")
                    for dt in range(ND):
                        yield lambda dt=dt, pp=pp: nc.tensor.matmul(
                            pp[:],
                            xT[:, dt, lt * 128:(lt + 1) * 128],
                            WTv[:, dt, :],
                            start=(dt == 0),
                            stop=(dt == ND - 1),
                        )
                    yield lambda pp=pp: nc.vector.tensor_copy(
                        Vaug[:, lt, :, 0:64], pp[:]
                    )
                return gen()

            def q_proj_group(j, qtile, et):
                def gen():
                    pp = psP.tile([128, 512], F32, tag="pp", name="pp")
                    for dt in range(ND):
                        yield lambda dt=dt, pp=pp: nc.tensor.matmul(
                            pp[:],
                            WTq[:, dt, et * 128:(et + 1) * 128],
                            xT[:, dt, j * 512:(j + 1) * 512],
                            start=(dt == 0),
                            stop=(dt == ND - 1),
                        )
                    yield lambda pp=pp: nc.vector.tensor_copy(qtile[:, et, :], pp[:])
                return gen()

            def out_proj_group(lt, ec):
                def gen():
                    op = psP.tile([128, 512], F32, tag="pp", name="op")
                    for dt in range(NDO):
                        yield lambda dt=dt, op=op: nc.tensor.matmul(
                            op[:],
                            attT[:, dt, lt * 128:(lt + 1) * 128],
                            WoT[:, dt, ec * 512:(ec + 1) * 512],
                            start=(dt == 0),
                            stop=(dt == NDO - 1),
                        )
                    def tail(op=op):
                        ot = smallp.tile([128, 512], F16, tag="ot", name="ot")
                        nc.vector.tensor_copy(ot[:], op[:])
                        nc.sync.dma_start(
                            out[lt * 128:(lt + 1) * 128, ec * 512:(ec + 1) * 512],
                            ot[:],
                        )
                    yield tail
                return gen()

            def chain(gens):
                for g in gens:
                    yield from g

            def drain(it, n):
                k = 0
                for f in it:
                    f()
                    k += 1
                    if k >= n:
                        return

            # ---------- one-head-deep AV/epilogue pipeline state ----------
            # pend = (j, h, PT, QTc) whose AV has not run yet
            state = {"pend": None, "att_pair": None}

            def av_steps(jh):
                """Yield closures: 4 AV accumulation groups, then the
                epilogue (reciprocal + normalize + pair transposes)."""
                j, h, PT = jh
                hp = (h % 2) * 64
                hb = h // 2
                if h % 2 == 0:
                    state["att_pair"] = attsbp.tile(
                        [128, 4, 128], MM, tag="apair", name="apair"
                    )
                att_pair = state["att_pair"]
                att_ps = psM.tile([128, 4, 65], F32, tag="m", name="att_ps")

                def group(qt):
                    nq = 4 * j + qt + 1
                    for kt in range(nq):
                        nc.tensor.matmul(
                            att_ps[:, qt, :],
                            PT[:, kt, qt * 128:(qt + 1) * 128],
                            Vaug[:, kt, h, 0:65],
                            start=(kt == 0),
                            stop=(kt == nq - 1),
                        )

                for qt in range(4):
                    yield lambda qt=qt: group(qt)

                def epilogue():
                    rc = smallp.tile([128, 4, 1], F32, tag="rc", name="rc")
                    nc.vector.reciprocal(rc[:], att_ps[:, :, 64:65])
                    nc.vector.tensor_mul(
                        att_pair[:, :, hp:hp + 64],
                        att_ps[:, :, 0:64],
                        rc[:, :, 0:1].to_broadcast((128, 4, 64)),
                    )
                    if h % 2 == 1:
                        for qt in range(4):
                            tpa = psM.tile([128, 128], F16, tag="m", name="tpa")
                            nc.tensor.transpose(
                                tpa[:], att_pair[:, qt, :], ident_h[:]
                            )
                            nc.vector.tensor_copy(
                                attT[:, hb, j * 512 + qt * 128:j * 512 + (qt + 1) * 128],
                                tpa[:],
                            )
                yield epilogue

            # ---------- prologue: just K/Q of chunk 0 for head-pair 0 ----------
            qtiles = {0: qtcp.tile([128, NE, 512], MM, tag="qt", name="qt0")}
            drain(k_proj_group(0, 0), 99)
            drain(q_proj_group(0, qtiles[0], 0), 99)

            # ---------- chunk loop ----------
            for j in range(NJ):
                gens = []
                watermarks = {}
                if j == 0:
                    # rest of chunk 0's own projections, interleaved so that
                    # head-pair hb's K/Q land just before its scores
                    for lt in range(4):
                        gens.append(v_proj_group(lt))
                    for et in range(1, NE):
                        gens.append(k_proj_group(et, 0))
                        gens.append(q_proj_group(0, qtiles[0], et))
                    watermarks = {1: 36, 2: 54, 4: 72, 6: 90}
                if j + 1 < NJ:
                    qtiles[j + 1] = qtcp.tile(
                        [128, NE, 512], MM, tag="qt", name=f"qt{j+1}"
                    )
                    for et in range(NE):
                        gens.append(k_proj_group(et, j + 1))
                    for et in range(NE):
                        gens.append(q_proj_group(j + 1, qtiles[j + 1], et))
                    for lt in range(4 * (j + 1), 4 * (j + 1) + 4):
                        gens.append(v_proj_group(lt))
                n_fill_items = (9 * 10 if j == 0 else 0) + (
                    9 * 12 if j + 1 < NJ else 0
                )
                op_gens = []
                if j == 1:
                    op_lts = range(0, 4)
                elif j == 3:
                    op_lts = range(4, 12)
                else:
                    op_lts = range(0)
                for lt in op_lts:
                    for ec in range(2):
                        op_gens.append(out_proj_group(lt, ec))
                n_op_items = 5 * len(op_gens)
                filler = chain(gens)
                op_filler = chain(op_gens)
                nkt = 4 * (j + 1)
                QTc = qtiles[j]
                n_slots = 8 * (nkt // 2)
                n_op_slots = 7 * (nkt // 2)
                fill_acc = 0.0
                fill_done = 0
                op_acc = 0.0
                op_done = 0
                for h in range(8):
                    hp = (h % 2) * 64
                    hb = h // 2
                    if h in watermarks:
                        need = watermarks[h] - fill_done
                        if need > 0:
                            drain(filler, need)
                            fill_done += need
                            fill_acc = max(fill_acc, float(fill_done))
                    PT = ptp.tile([128, NK, 512], MM, tag="pt", name="pt")
                    pend_steps = (
                        av_steps(state["pend"]) if state["pend"] is not None else None
                    )
                    n_pend = 5 if pend_steps is not None else 0
                    pend_acc = 0.0
                    pend_done = 0
                    slots = nkt // 2
                    for sl in range(slots):
                        s_ps = psS.tile([128, 2, 512], F32, tag="s", name="s_ps")
                        if sl >= 2:
                            # full-width k-tile pair (emitted after the diag
                            # pairs so the big exp tiles trail, keeping the
                            # scalar engine busy across the head boundary)
                            for half in range(2):
                                kt = 2 * (sl - 2) + half
                                nc.tensor.matmul(
                                    s_ps[:, half, :],
                                    KT[hp:hp + 64, hb, kt * 128:(kt + 1) * 128],
                                    QTc[hp:hp + 64, hb, :],
                                )
                            nc.scalar.activation(
                                PT[:, 2 * (sl - 2):2 * (sl - 2) + 2, :],
                                s_ps[:],
                                AF.Exp,
                            )
                        else:
                            # diagonal pair at 128-col causal granularity
                            mp = sl
                            for half in range(2):
                                m = 2 * mp + half
                                kt = nkt - 4 + m
                                nc.tensor.matmul(
                                    s_ps[:, half, 128 * m:512],
                                    KT[hp:hp + 64, hb, kt * 128:(kt + 1) * 128],
                                    QTc[hp:hp + 64, hb, 128 * m:512],
                                )
                            for half in range(2):
                                m = 2 * mp + half
                                kt = nkt - 4 + m
                                nc.scalar.activation(
                                    PT[:, kt, 128 * m:512],
                                    s_ps[:, half, 128 * m:512],
                                    AF.Exp,
                                )
                                nc.vector.tensor_mul(
                                    PT[:, kt, 128 * m:128 * (m + 1)],
                                    PT[:, kt, 128 * m:128 * (m + 1)],
                                    tri[:],
                                )
                        # interleave pending head's AV/epilogue
                        pend_acc += n_pend / slots
                        take = int(pend_acc) - pend_done
                        if take > 0 and pend_steps is not None:
                            drain(pend_steps, take)
                            pend_done += take
                        # smear next-chunk projection filler
                        fill_acc += n_fill_items / n_slots
                        take = int(fill_acc) - fill_done
                        if take > 0:
                            drain(filler, take)
                            fill_done += take
                        # output projection only once the cross-chunk pending
                        # head (which writes the last attT stripe) is done
                        if h >= 1:
                            op_acc += n_op_items / n_op_slots
                            take = int(op_acc) - op_done
                            if take > 0:
                                drain(op_filler, take)
                                op_done += take
                    if pend_steps is not None:
                        drain(pend_steps, 99)
                    state["pend"] = (j, h, PT)
                drain(filler, 10 ** 9)
                drain(op_filler, 10 ** 9)

            # final pending head + last chunk's output projection
            drain(av_steps(state["pend"]), 99)
            for lt in range(4 * (NJ - 1), 4 * (NJ - 1) + 4):
                for ec in range(2):
                    drain(out_proj_group(lt, ec), 99)

    nc.compile()
    return nc


def _get_program():
    if "nc" not in _CACHE:
        _CACHE["nc"] = build_program()
    return _CACHE["nc"]


def make_in_maps(x, Wq, Wk, Wv, Wo):
    x = np.asarray(x, dtype=np.float32)
    Wq = np.asarray(Wq, dtype=np.float32)
    Wk = np.asarray(Wk, dtype=np.float32)
    Wv = np.asarray(Wv, dtype=np.float32)
    Wo = np.asarray(Wo, dtype=np.float32)
    tri = (np.arange(128)[None, :] >= np.arange(128)[:, None]).astype(np.float16)
    in_maps = []
    for c in range(8):
        b, hg = c // 2, c % 2
        sl = slice(hg * E, (hg + 1) * E)
        in_maps.append(
            {
                "xT": x[b].T.astype(np.float16),
                "wqT": Wq[sl].T.astype(np.float16),
                "wkT": (Wk[sl] * 0.125).T.astype(np.float16),
                "wvT": Wv[sl].T.astype(np.float16),
                "woT": Wo[:, sl].T.astype(np.float16),
                "tri": tri,
            }
        )
    return in_maps


def kernel(x, Wq, Wk, Wv, Wo, **run_kwargs):
    from concourse import bass_utils

    nc = _get_program()
    in_maps = make_in_maps(x, Wq, Wk, Wv, Wo)
    res = bass_utils.run_bass_kernel_spmd(
        nc, in_maps, core_ids=list(range(8)), **run_kwargs
    )
    o = np.empty((B, L, D), np.float32)
    for b in range(B):
        o[b] = res.results[2 * b]["out"].astype(np.float32) + res.results[
            2 * b + 1
        ]["out"].astype(np.float32)
    _CACHE["last_result"] = res
    return o
